# revision 49
# baseline (speedup 1.0000x reference)
"""MoE (top-2 of 8 experts, d=1024, h=4096) on 8 Trainium2 NeuronCores.

Strategy (expert-parallel, per sharding hint):
  - Host: gating (fp64 logits/softmax/top-2 — tie margins on this problem are
    ~1e-5, far above fp32 rounding noise, so host routing matches the
    reference's fp32 top-k), per-expert token gather, pad to capacity C.
  - Device (core e = expert e): hidT = relu(W1_e.T @ x_e.T + b1_e) then
    ye = hidT.T @ W2_e, both as K-tiled 128x128x512 matmuls in float32r
    (full PE rate, ~1e-4 matmul rel err).
  - Host: out[tok_e] += gate_e * (ye + b2_e)  (scatter-combine).

Self-contained: hardcodes all shapes; only imports concourse (system lib).
"""

import os

os.environ.setdefault("JAX_PLATFORMS", "")

import numpy as np

import concourse.bacc as bacc
import concourse.mybir as mybir
import concourse.tile as tile
from concourse.bass_utils import run_bass_kernel_spmd

P = 128
D = 1024  # embed dim
H = 4096  # hidden dim
E = 8  # experts
TOPK = 2
KD = D // P  # 8  k-tiles over embed
KH = H // P  # 32 k-tiles over hidden
NCORES = 8
FD = 512  # matmul moving free dim (one PSUM bank of fp32)

_compiled = {}
LAST_RESULT = None  # BassKernelResults of the most recent run (for test harness)


def _phase1(nc, tc, rs, C, chunks, xt_d, w1_d, b1_d, hid_cs):
    f32 = mybir.dt.float32
    f32r = mybir.dt.float32r
    relu = mybir.ActivationFunctionType.Relu
    TN = len(chunks)
    with (
        tc.tile_pool(name=rs + "xts_p", bufs=1) as xts_p,
        tc.tile_pool(name=rs + "b1_p", bufs=1) as b1_p,
        tc.tile_pool(name=rs + "w1_p", bufs=int(os.environ.get("MOE_W1B", "4"))) as w1_p,
        tc.tile_pool(name=rs + "hb_p", bufs=1) as hb_p,
        tc.tile_pool(name=rs + "ps1", bufs=int(os.environ.get("MOE_PS1", "4")), space="PSUM") as ps1,
    ):
        def load_w1(hm):
            w1t = w1_p.tile([P, KD, P], f32r, tag="w1t", name=rs + f"w1t_{hm}")
            nc.sync.dma_start(w1t[:], w1_d[:, hm])
            return w1t

        # Issue order matters: HWDGE dispatches in program order, so the
        # first matmul group's deps (w1t[0] + x chunk 0) are issued first.
        w1_pre = {0: load_w1(0)}
        # x chunks as separate per-k tiles so the first matmul group only
        # waits on its own 8 pieces (~2MB), not the whole 10MB load.
        xc = [[None] * KD for _ in range(TN)]
        for tn, (off, w) in enumerate(chunks):
            for k0 in range(0, KD, 2):
                t = xts_p.tile(
                    [P, 2, w], f32r, tag=f"x_{tn}_{k0}", name=rs + f"x_{tn}_{k0}"
                )
                nc.sync.dma_start(t[:], xt_d[:, k0 : k0 + 2, off : off + w])
                xc[tn][k0] = t[:, 0, :]
                xc[tn][k0 + 1] = t[:, 1, :]
            if tn == 0:
                # b1 is first needed at the first eviction, not the first
                # matmul: issue it after chunk 0's loads.
                b1s = b1_p.tile([P, KH], f32, name=rs + "b1s")
                nc.sync.dma_start(b1s[:], b1_d[:])
            if tn < 3:  # prefetch next stationary tiles early
                w1_pre[tn + 1] = load_w1(tn + 1)
        # PE emission order: the first W hm rows are swept tn-major (wave
        # order) so the earliest matmuls only touch x chunks that have
        # already landed; the rest are hm-major. Each (hm, tn) psum group is
        # independent, so this only reorders work.
        W = int(os.environ.get("MOE_W", "2")) if TN > 1 else 0
        sched = [(hm, tn) for tn in range(TN) for hm in range(W)]
        sched += [(hm, tn) for hm in range(W, KH) for tn in range(TN)]

        w1ts, done = {}, {}
        KQ1 = KH // 4
        for hm, tn in sched:
            if hm not in w1ts:
                w1ts[hm] = w1_pre.pop(hm) if hm in w1_pre else load_w1(hm)
                done[hm] = 0
            off, w = chunks[tn]
            pt = ps1.tile([P, FD], f32, tag="ps1", name=rs + f"ps1_{hm}_{tn}")
            for k in range(KD):
                nc.tensor.matmul(
                    pt[:, :w],
                    w1ts[hm][:, k, :],
                    xc[tn][k],
                    start=(k == 0),
                    stop=(k == KD - 1),
                )
            # evict through a small per-chunk staging tile (ACT does
            # relu+bias, then the hid write DMAs it straight out on the ACT
            # HWDGE ring so phase-2 loads (SP ring) aren't queued behind it)
            hbst = int(os.environ.get("MOE_HBST", "12")) if C <= 2560 else 6
            hb = hb_p.tile([P, w], f32r, tag="hbst", bufs=hbst, name=rs + f"hb_{hm}_{tn}")
            nc.scalar.activation(
                hb[:, :w], pt[:, :w], relu, bias=b1s[:, hm : hm + 1]
            )
            nc.scalar.dma_start(
                hid_cs[tn][hm // KQ1][:, :, hm % KQ1, :].transpose([1, 0, 2]),
                hb.rearrange("p (t q) -> p t q", q=P),
            )
            done[hm] += 1
            if done[hm] == TN:
                del w1ts[hm]  # release references; pool slots recycle


W2HEAD = 8  # w2 chunks living in the persistent pool (loadable during phase 1)


def _phase2(nc, tc, rs, C, chunks, w2_d, hid_cs, ye_d, hd_p, w2h_p, ps2):
    f32 = mybir.dt.float32
    f32r = mybir.dt.float32r
    TM = C // P
    with (
        tc.tile_pool(name=rs + "w2_p", bufs=1) as w2_p,
        tc.tile_pool(name=rs + "out_p", bufs=int(os.environ.get("MOE_OUTB", "3"))) as out_p,
    ):

        HDS = 4  # hd k-split (must match the 4-way hid_cs DRAM split)
        KQ = KH // HDS

        def load_hd(tm):
            cidx = next(
                i for i, (off, w) in enumerate(chunks) if off // P <= tm < (off + w) // P
            )
            local = tm - chunks[cidx][0] // P
            parts = []
            for q in range(HDS):
                hdq = hd_p.tile(
                    [P, KQ, P], f32r, tag=f"hd{q}", name=rs + f"hd_{tm}_{q}"
                )
                nc.sync.dma_start(hdq[:], hid_cs[cidx][q][local])
                parts.append(hdq)
            return parts

        # Issue order: w2 head + first token tile's data before the bulk w2
        # load, so the first phase-2 matmul isn't queued behind 16MB of w2 on
        # the in-order HWDGE ring. Head w2 + hd live in pools hoisted outside
        # phase 1's, so these loads can run during phase 1's tail.
        w2ts = []
        for k in range(W2HEAD):
            w2t = w2h_p.tile([P, D], f32r, tag=f"w2_{k}", name=rs + f"w2_{k}")
            nc.sync.dma_start(w2t[:], w2_d[k])
            w2ts.append(w2t)
        hd_pre = {0: load_hd(0)}
        for k in range(W2HEAD, KH):
            w2t = w2_p.tile([P, D], f32r, tag=f"w2_{k}", name=rs + f"w2_{k}")
            nc.sync.dma_start(w2t[:], w2_d[k])
            w2ts.append(w2t)
            if k == 15:
                hd_pre[1] = load_hd(1)
        hd_pre[2] = load_hd(2)
        for tm in range(TM):
            hd = hd_pre.pop(tm) if tm in hd_pre else load_hd(tm)
            ob = out_p.tile([P, D], f32, tag="ob", name=rs + f"ob_{tm}")
            for n in range(D // FD):
                pt2 = ps2.tile([P, FD], f32, tag="ps2", name=rs + f"ps2_{tm}_{n}")
                for k in range(KH):
                    nc.tensor.matmul(
                        pt2[:],
                        hd[k // KQ][:, k % KQ, :],
                        w2ts[k][:, n * FD : (n + 1) * FD],
                        start=(k == 0),
                        stop=(k == KH - 1),
                    )
                nc.vector.tensor_copy(ob[:, n * FD : (n + 1) * FD], pt2[:])
            nc.scalar.dma_start(ye_d[tm], ob[:])


def _build(C, reps=1):
    """Per-core SPMD program for capacity-C tokens through one expert.

    reps>1 repeats the whole program back-to-back (timing experiments only).
    """
    if (C, reps) in _compiled:
        return _compiled[(C, reps)]
    f32 = mybir.dt.float32
    f32r = mybir.dt.float32r
    TM = C // P  # token tiles (GEMM2 stationary / output rows)
    # GEMM1 moving chunks: 512s plus one remainder (multiple of 128; N>=256
    # keeps fp32r at full rate, a 128 tail is negligible)
    chunks = []
    off = 0
    CW = int(os.environ.get("MOE_CW", "0"))
    if CW and C % CW == 0:  # uniform chunk-width experiment knob
        while off < C:
            chunks.append((off, CW))
            off += CW
    else:
        if C >= 768:  # small first chunk -> first matmul group starts sooner
            chunks.append((0, 256))
            off = 256
        while off < C:
            w = min(FD, C - off)
            chunks.append((off, w))
            off += w

    nc = bacc.Bacc(None, target_bir_lowering=False)
    # xt host layout [P, KD, C]: xt[p, k, c] = x[tok_c, k*128+p] (transposed)
    xt_d = nc.dram_tensor("xt", [P, KD, C], f32r, kind="ExternalInput")
    # w1 host layout [P, KH, KD, P]: w1[p, hm, k, j] = W1[k*128+p, hm*128+j]
    # -> per-hm stationary-tile loads are contiguous 4KB per partition.
    w1_d = nc.dram_tensor("w1", [P, KH, KD, P], f32r, kind="ExternalInput")
    b1_d = nc.dram_tensor("b1", [P, KH], f32, kind="ExternalInput")
    w2_d = nc.dram_tensor("w2", [KH, P, D], f32r, kind="ExternalInput")
    ye_d = nc.dram_tensor("ye", [TM, P, D], f32, kind="ExternalOutput")

    with tile.TileContext(nc) as tc:
        with tc.tile_pool(name="dram", bufs=1, space="DRAM") as dram:
            # hidT blocks: [token-tile, hidden-in-tile (partition), hm,
            # token-in-tile] -> phase-2 reads are contiguous 16KB/partition.
            # One DRAM tile per token chunk so phase-2's first loads only
            # depend on writes to their own chunk.
            # ... and per k-quarter, so phase-2's early hd quarters depend
            # only on the phase-1 rows that produced them (DRAM deps are
            # whole-tile).
            hid_cs = [
                [
                    dram.tile(
                        [w // P, P, KH // 4, P],
                        f32r,
                        tag=f"hidc_{i}_{q}",
                        name=f"hidc_{i}_{q}",
                    )
                    for q in range(4)
                ]
                for i, (off, w) in enumerate(chunks)
            ]
            for rep in range(reps):
                rs = "" if rep == 0 else f"r{rep}_"
                # hd/w2-head/psum2 pools are hoisted outside phase 1's pools
                # so phase 2's first loads don't wait for phase-1 SBUF release.
                with (
                    tc.tile_pool(name=rs + "hd_p", bufs=3) as hd_p,
                    tc.tile_pool(name=rs + "w2h_p", bufs=1) as w2h_p,
                    tc.tile_pool(name=rs + "ps2", bufs=int(os.environ.get("MOE_PS2", "4")), space="PSUM") as ps2,
                ):
                    _phase1(nc, tc, rs, C, chunks, xt_d, w1_d, b1_d, hid_cs)
                    _phase2(
                        nc, tc, rs, C, chunks, w2_d, hid_cs, ye_d, hd_p, w2h_p, ps2
                    )

    nc.compile()
    _compiled[(C, reps)] = nc
    return nc


def kernel(x, Wg, bg, W1, b1, W2, b2):
    global LAST_RESULT
    x = np.ascontiguousarray(x, dtype=np.float32)
    B, S, d = x.shape
    assert d == D
    T = B * S
    xf = x.reshape(T, d)

    # ---- Host gating/routing (fp64) ----
    logits = xf.astype(np.float64) @ Wg.astype(np.float64) + bg.astype(np.float64)
    mx = logits.max(axis=1, keepdims=True)
    ex = np.exp(logits - mx)
    probs = ex / ex.sum(axis=1, keepdims=True)
    order = np.argsort(-logits, axis=1, kind="stable")  # ties -> lower index
    top = order[:, :TOPK]  # [T, 2]
    gsel = np.take_along_axis(probs, top, axis=1).astype(np.float32)

    toks, gates = [], []
    for e in range(E):
        pos = top == e  # [T, 2]
        sel = pos.any(axis=1)
        toks.append(np.nonzero(sel)[0])
        gates.append((gsel * pos).sum(axis=1)[sel].astype(np.float32))

    maxcnt = max(len(t) for t in toks)
    # SBUF budget caps resident x at 4096 tokens/core; batch if routing is
    # ever concentrated enough to exceed that (never for balanced gating).
    MAXC = 2944
    nb = max(1, -(-maxcnt // MAXC))
    C = max(P, ((-(-maxcnt // nb) + P - 1) // P) * P)

    w_maps = []  # per-expert weight shards (batch-invariant)
    for e in range(E):
        w_maps.append(
            {
                "w1": np.ascontiguousarray(
                    np.asarray(W1[e], dtype=np.float32)
                    .reshape(KD, P, KH, P)
                    .transpose(1, 2, 0, 3)
                ),
                "b1": np.ascontiguousarray(
                    np.asarray(b1[e], dtype=np.float32).reshape(KH, P).T
                ),
                "w2": np.ascontiguousarray(W2[e], dtype=np.float32).reshape(KH, P, D),
            }
        )

    nc = _build(C)
    out = np.zeros((T, D), np.float32)
    b2f = np.asarray(b2, dtype=np.float32)
    for b in range(nb):
        in_maps = []
        btoks = []
        for e in range(E):
            tk = toks[e][b * C : (b + 1) * C]
            btoks.append(tk)
            xe = np.zeros((C, D), np.float32)
            xe[: len(tk)] = xf[tk]
            in_maps.append(
                {
                    "xt": np.ascontiguousarray(
                        xe.T.reshape(KD, P, C).transpose(1, 0, 2)
                    ),
                    **w_maps[e],
                }
            )
        res = run_bass_kernel_spmd(nc, in_maps, core_ids=list(range(NCORES)))
        LAST_RESULT = res
        for e in range(E):
            cnt = len(btoks[e])
            if cnt == 0:
                continue
            ye = res.results[e]["ye"].reshape(C, D)[:cnt]
            g = gates[e][b * C : b * C + cnt]
            out[btoks[e]] += g[:, None] * (ye + b2f[e])
    return out.reshape(B, S, D)
